# revision 6
# baseline (speedup 1.0000x reference)
"""Causal self-attention with RoPE on 8 TRN2 NeuronCores.

Sharding: pure data parallel over batch B=8 (one batch element per core,
weights replicated, no collectives).

Per-core dataflow (everything "transposed" so softmax reductions and biases
land on friendly axes):
  xT = x^T                          via PE transpose          [C, T]
  q^T,k^T = W_qk^T @ x + b          PE (W stationary)         [ch, T]
  v natural = x @ W_v + b_v         PE (xT stationary)        [T, ch]
  RoPE(q,k)                         PE rotation matmul + DVE  in place
  s^T = k @ q^T (per head)          PE, K=64                  [Tk, Tq]
  p = exp(s/8) * causal_mask        ACT exp + DVE mask
  [y'; r]^T = [v, 1]^T @ p          PE, K=128 accumulation    [65, Tq]
  y^T = y'^T * (1/r)                DVE (+ gpsimd broadcast)
  out = y @ W_proj + b              PE (yT stationary)        [T, C]

Matmuls run in float32r (fp32 data, 12-bit-mantissa multiply) = 4x fp32 rate.
"""
import sys

sys.path.insert(0, "/opt/trn_rl_repo")

import numpy as np

B, T, C = 8, 1024, 768
H, D = 12, 64
N_CORES = 8
KC = C // 128  # 6 K-chunks of the C contraction
NT = T // 128  # 8 T-chunks

_prog = None  # cached compiled Bass program


def _emit_body(nc, tc, dr):
    """Emit one full forward pass. dr = dict of DRAM tensors."""
    from concourse import mybir

    F32 = mybir.dt.float32
    F32R = mybir.dt.float32r
    AFT = mybir.ActivationFunctionType

    with (
        tc.tile_pool(name="persist", bufs=1) as pp,
    ):
        # persistent tensors
        qkT = pp.tile([128, 12, T], F32R, tag="qkT")  # 0-5: q pairs, 6-11: k pairs
        v_sb = pp.tile([128, NT, H, 65], F32R, tag="v")  # v natural + ones col
        yT = pp.tile([128, KC, T], F32R, tag="yT")
        cos_sb = pp.tile([128, T], F32, tag="cos")
        sin_sb = pp.tile([128, T], F32, tag="sin")
        rt_sb = pp.tile([128, 128], F32R, tag="rt")
        idn_sb = pp.tile([128, 128], F32, tag="idn")
        mask_sb = pp.tile([128, 128], F32R, tag="mask")
        bqk_sb = pp.tile([128, 12], F32, tag="bqk")
        bv_sb = pp.tile([1, C], F32R, tag="bv")
        bp_sb = pp.tile([1, C], F32R, tag="bp")
        ones_sb = pp.tile([1, 128], F32R, tag="ones")

        nc.sync.dma_start(out=cos_sb[:], in_=dr["cosT"][:])
        nc.sync.dma_start(out=sin_sb[:], in_=dr["sinT"][:])
        nc.sync.dma_start(out=rt_sb[:], in_=dr["rt"][:].bitcast(F32R))
        nc.sync.dma_start(out=idn_sb[:], in_=dr["idn"][:])
        nc.sync.dma_start(out=mask_sb[:], in_=dr["mask"][:].bitcast(F32R))
        nc.sync.dma_start(out=bqk_sb[:], in_=dr["bqk"][:])
        nc.sync.dma_start(out=bv_sb[:], in_=dr["bv"][:].bitcast(F32R))
        nc.sync.dma_start(out=bp_sb[:], in_=dr["bp"][:].bitcast(F32R))
        onesF = pp.tile([128, 128], F32, tag="onesF")
        nc.vector.memset(onesF[:], 1.0)
        nc.vector.tensor_copy(ones_sb[:], onesF[0:1, :])
        for t in range(NT):
            nc.vector.tensor_copy(
                v_sb[:, t, :, 64:65],
                onesF[:, 0:12].rearrange("p (h o) -> p h o", h=12),
            )

        # ---------------- Phase A: transpose x, qkv, rope ----------------
        with (
            tc.tile_pool(name="pa_sb", bufs=2) as pa,
            tc.tile_pool(name="pa_xt", bufs=1) as paxt,
            tc.tile_pool(name="pa_ps", bufs=2, space="PSUM") as pap,
            tc.tile_pool(name="pa_mm", bufs=3, space="PSUM") as pam,
            tc.tile_pool(name="pa_tmp", bufs=4) as pat,
        ):
            xT = paxt.tile([128, KC, T], F32R, tag="xT")
            # load x in 128-row chunks, PE-transpose each [128,128] block
            for t in range(NT):
                xn = pa.tile([128, C], F32, tag="xn")
                nc.sync.dma_start(out=xn[:], in_=dr["x"][t * 128 : (t + 1) * 128, :])
                for c in range(KC):
                    ptr = pap.tile([128, 128], F32, tag="tr")
                    nc.tensor.transpose(
                        ptr[:], xn[:, c * 128 : (c + 1) * 128], idn_sb[:]
                    )
                    nc.scalar.activation(
                        xT[:, c, t * 128 : (t + 1) * 128], ptr[:], AFT.Copy
                    )

            # qkv in 6 column groups of 384 (W_attn streamed per group)
            wa_r = dr["wa"][:].bitcast(F32R).rearrange("(kc p) n -> p kc n", p=128)
            for g in range(6):
                wt = pa.tile([128, KC, 384], F32R, tag="walt")
                nc.sync.dma_start(out=wt[:], in_=wa_r[:, :, g * 384 : (g + 1) * 384])
                if g < 4:  # q/k output chunks m = 3g..3g+2
                    for mi in range(3):
                        m = 3 * g + mi
                        for pj in range(2):
                            w = slice(pj * 512, (pj + 1) * 512)
                            ps = pam.tile([128, 512], F32, tag="mm")
                            for kc in range(KC):
                                nc.tensor.matmul(
                                    ps[:],
                                    wt[:, kc, mi * 128 : (mi + 1) * 128],
                                    xT[:, kc, w],
                                    start=(kc == 0),
                                    stop=(kc == KC - 1),
                                )
                            nc.scalar.activation(
                                qkT[:, m, w],
                                ps[:],
                                AFT.Identity,
                                bias=bqk_sb[:, m : m + 1],
                            )
                else:  # v columns: 384-wide piece covers 6 heads
                    vg = g - 4
                    h0 = 6 * vg
                    for t in range(NT):
                        ps = pam.tile([128, 384], F32, tag="mm")
                        for kc in range(KC):
                            nc.tensor.matmul(
                                ps[:],
                                xT[:, kc, t * 128 : (t + 1) * 128],
                                wt[:, kc, :],
                                start=(kc == 0),
                                stop=False,
                            )
                        nc.tensor.matmul(
                            ps[:],
                            ones_sb[:],
                            bv_sb[:, vg * 384 : (vg + 1) * 384],
                            start=False,
                            stop=True,
                        )
                        nc.scalar.activation(
                            v_sb[:, t, h0 : h0 + 6, 0:64],
                            ps[:].rearrange("p (h d) -> p h d", h=6),
                            AFT.Copy,
                        )

            # RoPE in place on all 12 q/k tiles
            for i in range(12):
                for pj in range(2):
                    w = slice(pj * 512, (pj + 1) * 512)
                    rp = pap.tile([128, 512], F32, tag="rot")
                    nc.tensor.matmul(
                        rp[:], rt_sb[:], qkT[:, i, w], start=True, stop=True
                    )
                    t1 = pat.tile([128, 512], F32, tag="t1")
                    nc.vector.tensor_mul(t1[:], qkT[:, i, w], cos_sb[:, w])
                    t2 = pat.tile([128, 512], F32, tag="t2")
                    nc.vector.tensor_mul(t2[:], rp[:], sin_sb[:, w])
                    nc.vector.tensor_add(qkT[:, i, w], t1[:], t2[:])

        # ---------------- Phase B: attention per head ----------------
        with (
            tc.tile_pool(name="pb_es", bufs=8) as pbe,
            tc.tile_pool(name="pb_sc", bufs=3) as pbs,
            tc.tile_pool(name="pb_st", bufs=3, space="PSUM") as pbst,
            tc.tile_pool(name="pb_yp", bufs=2, space="PSUM") as pbyp,
        ):
            for hp in range(6):
                for hh in range(2):
                    h = 2 * hp + hh
                    b0 = 64 * hh
                    qv, kv = hp, 6 + hp
                    for pj in range(2):
                        w0 = 512 * pj
                        tkcs = [k for k in range(NT) if 128 * k < w0 + 512]
                        es_list = []
                        for tkc in tkcs:
                            lo = max(w0, 128 * tkc)
                            wdt = w0 + 512 - lo
                            st = pbst.tile([128, 512], F32, tag="st")
                            nc.tensor.matmul(
                                st[:, :wdt],
                                qkT[b0 : b0 + 64, kv, tkc * 128 : (tkc + 1) * 128],
                                qkT[b0 : b0 + 64, qv, lo : lo + wdt],
                                start=True,
                                stop=True,
                            )
                            es = pbe.tile([128, 512], F32R, tag="es")
                            nc.scalar.activation(
                                es[:, :wdt], st[:, :wdt], AFT.Exp, scale=0.125
                            )
                            if lo == 128 * tkc:  # diagonal block: causal mask
                                nc.vector.tensor_mul(
                                    es[:, 0:128], es[:, 0:128], mask_sb[:]
                                )
                            es_list.append((tkc, es, lo, wdt))
                        yp = pbyp.tile([65, 512], F32, tag="yp")
                        for j, (tkc, es, lo, wdt) in enumerate(es_list):
                            nc.tensor.matmul(
                                yp[:, lo - w0 : lo - w0 + wdt],
                                v_sb[:, tkc, h, :],
                                es[:, :wdt],
                                start=(j == 0),
                                stop=(j == len(es_list) - 1),
                            )
                        # normalize: y = y' / r  (r = row 64 of yp)
                        rs = pbs.tile([128, 512], F32, tag="rs")
                        nc.vector.tensor_copy(rs[64:65, :], yp[64:65, :])
                        rb0 = pbs.tile([1, 512], F32, tag="rb0")
                        nc.sync.dma_start(out=rb0[:], in_=rs[64:65, :])
                        nc.vector.reciprocal(rb0[:], rb0[:])
                        rb = pbs.tile([64, 512], F32, tag="rb")
                        nc.gpsimd.partition_broadcast(rb[:], rb0[:])
                        if hh == 0:
                            nc.vector.tensor_mul(
                                yT[0:64, hp, w0 : w0 + 512], yp[0:64, :], rb[:]
                            )
                        else:
                            ys = pbs.tile([64, 512], F32R, tag="ys")
                            nc.vector.tensor_mul(ys[:], yp[0:64, :], rb[:])
                            nc.sync.dma_start(
                                out=yT[64:128, hp, w0 : w0 + 512], in_=ys[:]
                            )

        # ---------------- Phase C: output projection ----------------
        with (
            tc.tile_pool(name="pc_sb", bufs=1) as pcs,
            tc.tile_pool(name="pc_ob", bufs=3) as pco,
            tc.tile_pool(name="pc_ps", bufs=3, space="PSUM") as pcp,
        ):
            wp = pcs.tile([128, KC, C], F32R, tag="wp")
            nc.sync.dma_start(
                out=wp[:],
                in_=dr["wp"][:].bitcast(F32R).rearrange("(kc p) n -> p kc n", p=128),
            )
            for m in range(NT):
                osb = pco.tile([128, C], F32, tag="ob")
                for piece in range(2):
                    pw = slice(piece * 384, (piece + 1) * 384)
                    po = pcp.tile([128, 384], F32, tag="po")
                    for kc in range(KC):
                        nc.tensor.matmul(
                            po[:],
                            yT[:, kc, m * 128 : (m + 1) * 128],
                            wp[:, kc, pw],
                            start=(kc == 0),
                            stop=False,
                        )
                    nc.tensor.matmul(
                        po[:], ones_sb[:], bp_sb[:, pw], start=False, stop=True
                    )
                    nc.vector.tensor_copy(osb[:, pw], po[:])
                nc.sync.dma_start(out=dr["out"][m * 128 : (m + 1) * 128, :], in_=osb[:])


def _build_program(loop_n=None):
    import concourse.bacc as bacc
    import concourse.tile as tile
    from concourse import mybir

    F32 = mybir.dt.float32

    nc = bacc.Bacc(None, target_bir_lowering=False, debug=False)

    dr = {
        "x": nc.dram_tensor("x", [T, C], F32, kind="ExternalInput"),
        "wa": nc.dram_tensor("wa", [C, 3 * C], F32, kind="ExternalInput"),
        "bqk": nc.dram_tensor("bqk", [128, 12], F32, kind="ExternalInput"),
        "bv": nc.dram_tensor("bv", [1, C], F32, kind="ExternalInput"),
        "wp": nc.dram_tensor("wp", [C, C], F32, kind="ExternalInput"),
        "bp": nc.dram_tensor("bp", [1, C], F32, kind="ExternalInput"),
        "cosT": nc.dram_tensor("cosT", [128, T], F32, kind="ExternalInput"),
        "sinT": nc.dram_tensor("sinT", [128, T], F32, kind="ExternalInput"),
        "rt": nc.dram_tensor("rt", [128, 128], F32, kind="ExternalInput"),
        "idn": nc.dram_tensor("idn", [128, 128], F32, kind="ExternalInput"),
        "mask": nc.dram_tensor("mask", [128, 128], F32, kind="ExternalInput"),
        "out": nc.dram_tensor("out", [T, C], F32, kind="ExternalOutput"),
    }

    with tile.TileContext(nc) as tc:
        if loop_n is None:
            _emit_body(nc, tc, dr)
        else:
            with tc.For_i(0, loop_n, 1):
                _emit_body(nc, tc, dr)

    nc.compile()
    return nc


def _host_constants():
    """Constant tables shipped to every core."""
    inv_freq = (1.0 / (10000.0 ** (np.arange(0, D, 2, dtype=np.float32) / D))).astype(
        np.float32
    )
    tpos = np.arange(T, dtype=np.float32)
    freqs = tpos[None, :] * inv_freq[:, None]  # [32, T]
    cos32 = np.cos(freqs).astype(np.float32)
    sin32 = np.sin(freqs).astype(np.float32)
    cosT = np.repeat(cos32, 2, axis=0)  # [64, T], channel d -> freq d//2
    sinT = np.repeat(sin32, 2, axis=0)
    cosT = np.concatenate([cosT, cosT], axis=0)  # [128, T]: two head copies
    sinT = np.concatenate([sinT, sinT], axis=0)

    # rotation matrix: rot = R @ q with rot[2i] = -q[2i+1], rot[2i+1] = q[2i]
    R = np.zeros((128, 128), dtype=np.float32)
    idx = np.arange(0, 128, 2)
    R[idx, idx + 1] = -1.0
    R[idx + 1, idx] = 1.0
    RT = np.ascontiguousarray(R.T)

    idn = np.eye(128, dtype=np.float32)
    # causal mask in s^T orientation: keep tq_rel >= tk (upper incl diag)
    mask = np.triu(np.ones((128, 128), dtype=np.float32))
    return cosT, sinT, RT, idn, mask


def _input_maps(x, W_attn, b_attn, W_proj, b_proj):
    cosT, sinT, RT, idn, mask = _host_constants()
    shared = {
        "wa": np.ascontiguousarray(W_attn),
        "bqk": np.ascontiguousarray(b_attn[: 2 * C].reshape(12, 128).T),
        "bv": np.ascontiguousarray(b_attn[2 * C :].reshape(1, C)),
        "wp": np.ascontiguousarray(W_proj),
        "bp": np.ascontiguousarray(b_proj.reshape(1, C)),
        "cosT": cosT,
        "sinT": sinT,
        "rt": RT,
        "idn": idn,
        "mask": mask,
    }
    return [dict(shared, x=np.ascontiguousarray(x[b])) for b in range(B)]


def kernel(x, W_attn, b_attn, W_proj, b_proj):
    global _prog
    from concourse.bass_utils import run_bass_kernel_spmd

    if _prog is None:
        _prog = _build_program()

    x = np.asarray(x, dtype=np.float32)
    W_attn = np.asarray(W_attn, dtype=np.float32)
    b_attn = np.asarray(b_attn, dtype=np.float32)
    W_proj = np.asarray(W_proj, dtype=np.float32)
    b_proj = np.asarray(b_proj, dtype=np.float32)

    in_maps = _input_maps(x, W_attn, b_attn, W_proj, b_proj)
    res = run_bass_kernel_spmd(_prog, in_maps, list(range(N_CORES)))
    out = np.stack([res.results[b]["out"] for b in range(B)], axis=0)
    return out.astype(np.float32)


# revision 8
# speedup vs baseline: 1.7881x; 1.7881x over previous
"""Causal self-attention with RoPE on 8 TRN2 NeuronCores.

Sharding: pure data parallel over batch B=8 (one batch element per core,
weights replicated, no collectives).

Per-core dataflow (everything "transposed" so softmax reductions and biases
land on friendly axes):
  xT = x^T                          via PE transpose          [C, T]
  q^T,k^T = W_qk^T @ x + b          PE (W stationary)         [ch, T]
  v natural = x @ W_v + b_v         PE (xT stationary)        [T, ch]
  RoPE(q,k)                         PE rotation matmul + DVE  in place
  s^T = k @ q^T (per head)          PE, K=64                  [Tk, Tq]
  p = exp(s/8) * causal_mask        ACT exp + DVE mask
  [y'; r]^T = [v, 1]^T @ p          PE, K=128 accumulation    [65, Tq]
  y^T = y'^T * (1/r)                DVE (+ gpsimd broadcast)
  out = y @ W_proj + b              PE (yT stationary)        [T, C]

Matmuls run in float32r (fp32 data, 12-bit-mantissa multiply) = 4x fp32 rate.
"""
import sys

sys.path.insert(0, "/opt/trn_rl_repo")

import numpy as np

B, T, C = 8, 1024, 768
H, D = 12, 64
N_CORES = 8
KC = C // 128  # 6 K-chunks of the C contraction
NT = T // 128  # 8 T-chunks

_prog = None  # cached compiled Bass program


def _emit_body(nc, tc, dr):
    """Emit one full forward pass. dr = dict of DRAM tensors."""
    from concourse import mybir

    F32 = mybir.dt.float32
    F32R = mybir.dt.float32r
    AFT = mybir.ActivationFunctionType

    with (
        tc.tile_pool(name="persist", bufs=1) as pp,
    ):
        # persistent tensors
        qkT = pp.tile([128, 12, T], F32R, tag="qkT")  # 0-5: q pairs, 6-11: k pairs
        v_sb = pp.tile([128, NT, H, 65], F32R, tag="v")  # v natural + ones col
        yT = pp.tile([128, KC, T], F32R, tag="yT")
        cos_sb = pp.tile([128, T], F32, tag="cos")
        sin_sb = pp.tile([128, T], F32, tag="sin")
        rt_sb = pp.tile([128, 128], F32R, tag="rt")
        idn_sb = pp.tile([128, 128], F32, tag="idn")
        mask_sb = pp.tile([128, 128], F32R, tag="mask")
        bqk_sb = pp.tile([128, 12], F32, tag="bqk")
        bv_sb = pp.tile([1, C], F32R, tag="bv")
        bp_sb = pp.tile([1, C], F32R, tag="bp")
        ones_sb = pp.tile([1, 128], F32R, tag="ones")

        nc.sync.dma_start(out=cos_sb[:], in_=dr["cosT"][:])
        nc.sync.dma_start(out=sin_sb[:], in_=dr["sinT"][:])
        nc.sync.dma_start(out=rt_sb[:], in_=dr["rt"][:].bitcast(F32R))
        nc.sync.dma_start(out=idn_sb[:], in_=dr["idn"][:])
        nc.sync.dma_start(out=mask_sb[:], in_=dr["mask"][:].bitcast(F32R))
        nc.sync.dma_start(out=bqk_sb[:], in_=dr["bqk"][:])
        nc.sync.dma_start(out=bv_sb[:], in_=dr["bv"][:].bitcast(F32R))
        nc.sync.dma_start(out=bp_sb[:], in_=dr["bp"][:].bitcast(F32R))
        onesF = pp.tile([128, 128], F32, tag="onesF")
        nc.vector.memset(onesF[:], 1.0)
        nc.vector.tensor_copy(ones_sb[:], onesF[0:1, :])
        for t in range(NT):
            nc.vector.tensor_copy(
                v_sb[:, t, :, 64:65],
                onesF[:, 0:12].rearrange("p (h o) -> p h o", h=12),
            )

        # ---------------- Phase A: transpose x, qkv, rope ----------------
        with (
            tc.tile_pool(name="pa_sb", bufs=2) as pa,
            tc.tile_pool(name="pa_xt", bufs=1) as paxt,
            tc.tile_pool(name="pa_ps", bufs=2, space="PSUM") as pap,
            tc.tile_pool(name="pa_mm", bufs=3, space="PSUM") as pam,
            tc.tile_pool(name="pa_tmp", bufs=3) as pat,
        ):
            xT = paxt.tile([128, KC, T], F32R, tag="xT")
            # load x in 128-row chunks, PE-transpose each [128,128] block
            for t in range(NT):
                xn = pa.tile([128, C], F32, tag="xn")
                nc.sync.dma_start(out=xn[:], in_=dr["x"][t * 128 : (t + 1) * 128, :])
                for c in range(KC):
                    ptr = pap.tile([128, 128], F32, tag="tr")
                    nc.tensor.transpose(
                        ptr[:], xn[:, c * 128 : (c + 1) * 128], idn_sb[:]
                    )
                    nc.scalar.activation(
                        xT[:, c, t * 128 : (t + 1) * 128], ptr[:], AFT.Identity
                    )

            # qkv in 6 column groups of 384 (W_attn streamed per group)
            wa_r = dr["wa"][:].bitcast(F32R).rearrange("(kc p) n -> p kc n", p=128)
            for g in range(6):
                wt = pa.tile([128, KC, 384], F32R, tag="walt")
                nc.sync.dma_start(out=wt[:], in_=wa_r[:, :, g * 384 : (g + 1) * 384])
                if g < 4:  # q/k output chunks m = 3g..3g+2
                    for mi in range(3):
                        m = 3 * g + mi
                        for pj in range(2):
                            w = slice(pj * 512, (pj + 1) * 512)
                            ps = pam.tile([128, 512], F32, tag="mm")
                            for kc in range(KC):
                                nc.tensor.matmul(
                                    ps[:],
                                    wt[:, kc, mi * 128 : (mi + 1) * 128],
                                    xT[:, kc, w],
                                    start=(kc == 0),
                                    stop=(kc == KC - 1),
                                )
                            nc.scalar.activation(
                                qkT[:, m, w],
                                ps[:],
                                AFT.Identity,
                                bias=bqk_sb[:, m : m + 1],
                            )
                else:  # v columns: 384-wide piece covers 6 heads
                    vg = g - 4
                    h0 = 6 * vg
                    for t in range(NT):
                        ps = pam.tile([128, 384], F32, tag="mm")
                        for kc in range(KC):
                            nc.tensor.matmul(
                                ps[:],
                                xT[:, kc, t * 128 : (t + 1) * 128],
                                wt[:, kc, :],
                                start=(kc == 0),
                                stop=False,
                            )
                        nc.tensor.matmul(
                            ps[:],
                            ones_sb[:],
                            bv_sb[:, vg * 384 : (vg + 1) * 384],
                            start=False,
                            stop=True,
                        )
                        nc.scalar.activation(
                            v_sb[:, t, h0 : h0 + 6, 0:64],
                            ps[:].rearrange("p (h d) -> p h d", h=6),
                            AFT.Identity,
                        )

            # RoPE in place on all 12 q/k tiles
            for i in range(12):
                t1 = pat.tile([128, T], F32, tag="t1")
                nc.vector.tensor_mul(t1[:], qkT[:, i, :], cos_sb[:])
                for pj in range(2):
                    w = slice(pj * 512, (pj + 1) * 512)
                    rp = pap.tile([128, 512], F32, tag="rot")
                    nc.tensor.matmul(
                        rp[:], rt_sb[:], qkT[:, i, w], start=True, stop=True
                    )
                    t2 = pat.tile([128, 512], F32, tag="t2")
                    nc.vector.tensor_mul(t2[:], rp[:], sin_sb[:, w])
                    nc.vector.tensor_add(qkT[:, i, w], t1[:, w], t2[:])

        # ---------------- Phase B: attention per head ----------------
        with (
            tc.tile_pool(name="pb_es", bufs=8) as pbe,
            tc.tile_pool(name="pb_sc", bufs=3) as pbs,
            tc.tile_pool(name="pb_st", bufs=4, space="PSUM") as pbst,
            tc.tile_pool(name="pb_yp", bufs=4, space="PSUM") as pbyp,
        ):
            for hp in range(6):
                for hh in range(2):
                    h = 2 * hp + hh
                    b0 = 64 * hh
                    qv, kv = hp, 6 + hp
                    for pj in range(2):
                        w0 = 512 * pj
                        tkcs = [k for k in range(NT) if 128 * k < w0 + 512]
                        es_list = []
                        for tkc in tkcs:
                            lo = max(w0, 128 * tkc)
                            wdt = w0 + 512 - lo
                            st = pbst.tile([128, 512], F32, tag="st")
                            nc.tensor.matmul(
                                st[:, :wdt],
                                qkT[b0 : b0 + 64, kv, tkc * 128 : (tkc + 1) * 128],
                                qkT[b0 : b0 + 64, qv, lo : lo + wdt],
                                start=True,
                                stop=True,
                            )
                            es = pbe.tile([128, 512], F32R, tag="es")
                            nc.scalar.activation(
                                es[:, :wdt], st[:, :wdt], AFT.Exp, scale=0.125
                            )
                            if lo == 128 * tkc:  # diagonal block: causal mask
                                nc.vector.tensor_mul(
                                    es[:, 0:128], es[:, 0:128], mask_sb[:]
                                )
                            es_list.append((tkc, es, lo, wdt))
                        yp = pbyp.tile([65, 512], F32, tag="yp")
                        for j, (tkc, es, lo, wdt) in enumerate(es_list):
                            nc.tensor.matmul(
                                yp[:, lo - w0 : lo - w0 + wdt],
                                v_sb[:, tkc, h, :],
                                es[:, :wdt],
                                start=(j == 0),
                                stop=(j == len(es_list) - 1),
                            )
                        # normalize: y = y' / r  (r = row 64 of yp)
                        rs = pbs.tile([128, 512], F32, tag="rs")
                        nc.vector.tensor_copy(rs[64:65, :], yp[64:65, :])
                        rb0 = pbs.tile([1, 512], F32, tag="rb0")
                        nc.sync.dma_start(out=rb0[:], in_=rs[64:65, :])
                        nc.vector.reciprocal(rb0[:], rb0[:])
                        rb = pbs.tile([64, 512], F32, tag="rb")
                        nc.gpsimd.partition_broadcast(rb[:], rb0[:])
                        if hh == 0:
                            nc.vector.tensor_mul(
                                yT[0:64, hp, w0 : w0 + 512], yp[0:64, :], rb[:]
                            )
                        else:
                            ys = pbs.tile([64, 512], F32R, tag="ys")
                            nc.vector.tensor_mul(ys[:], yp[0:64, :], rb[:])
                            nc.sync.dma_start(
                                out=yT[64:128, hp, w0 : w0 + 512], in_=ys[:]
                            )

        # ---------------- Phase C: output projection ----------------
        with (
            tc.tile_pool(name="pc_sb", bufs=1) as pcs,
            tc.tile_pool(name="pc_ob", bufs=3) as pco,
            tc.tile_pool(name="pc_ps", bufs=3, space="PSUM") as pcp,
        ):
            wp = pcs.tile([128, KC, C], F32R, tag="wp")
            nc.sync.dma_start(
                out=wp[:],
                in_=dr["wp"][:].bitcast(F32R).rearrange("(kc p) n -> p kc n", p=128),
            )
            for m in range(NT):
                osb = pco.tile([128, C], F32, tag="ob")
                for piece in range(2):
                    pw = slice(piece * 384, (piece + 1) * 384)
                    po = pcp.tile([128, 384], F32, tag="po")
                    for kc in range(KC):
                        nc.tensor.matmul(
                            po[:],
                            yT[:, kc, m * 128 : (m + 1) * 128],
                            wp[:, kc, pw],
                            start=(kc == 0),
                            stop=False,
                        )
                    nc.tensor.matmul(
                        po[:], ones_sb[:], bp_sb[:, pw], start=False, stop=True
                    )
                    nc.vector.tensor_copy(osb[:, pw], po[:])
                nc.sync.dma_start(out=dr["out"][m * 128 : (m + 1) * 128, :], in_=osb[:])


def _build_program(loop_n=None):
    import concourse.bacc as bacc
    import concourse.tile as tile
    from concourse import mybir

    F32 = mybir.dt.float32

    nc = bacc.Bacc(None, target_bir_lowering=False, debug=False)

    dr = {
        "x": nc.dram_tensor("x", [T, C], F32, kind="ExternalInput"),
        "wa": nc.dram_tensor("wa", [C, 3 * C], F32, kind="ExternalInput"),
        "bqk": nc.dram_tensor("bqk", [128, 12], F32, kind="ExternalInput"),
        "bv": nc.dram_tensor("bv", [1, C], F32, kind="ExternalInput"),
        "wp": nc.dram_tensor("wp", [C, C], F32, kind="ExternalInput"),
        "bp": nc.dram_tensor("bp", [1, C], F32, kind="ExternalInput"),
        "cosT": nc.dram_tensor("cosT", [128, T], F32, kind="ExternalInput"),
        "sinT": nc.dram_tensor("sinT", [128, T], F32, kind="ExternalInput"),
        "rt": nc.dram_tensor("rt", [128, 128], F32, kind="ExternalInput"),
        "idn": nc.dram_tensor("idn", [128, 128], F32, kind="ExternalInput"),
        "mask": nc.dram_tensor("mask", [128, 128], F32, kind="ExternalInput"),
        "out": nc.dram_tensor("out", [T, C], F32, kind="ExternalOutput"),
    }

    with tile.TileContext(nc) as tc:
        if loop_n is None:
            _emit_body(nc, tc, dr)
        else:
            with tc.For_i(0, loop_n, 1):
                _emit_body(nc, tc, dr)

    nc.compile()
    return nc


def _host_constants():
    """Constant tables shipped to every core."""
    inv_freq = (1.0 / (10000.0 ** (np.arange(0, D, 2, dtype=np.float32) / D))).astype(
        np.float32
    )
    tpos = np.arange(T, dtype=np.float32)
    freqs = tpos[None, :] * inv_freq[:, None]  # [32, T]
    cos32 = np.cos(freqs).astype(np.float32)
    sin32 = np.sin(freqs).astype(np.float32)
    cosT = np.repeat(cos32, 2, axis=0)  # [64, T], channel d -> freq d//2
    sinT = np.repeat(sin32, 2, axis=0)
    cosT = np.concatenate([cosT, cosT], axis=0)  # [128, T]: two head copies
    sinT = np.concatenate([sinT, sinT], axis=0)

    # rotation matrix: rot = R @ q with rot[2i] = -q[2i+1], rot[2i+1] = q[2i]
    R = np.zeros((128, 128), dtype=np.float32)
    idx = np.arange(0, 128, 2)
    R[idx, idx + 1] = -1.0
    R[idx + 1, idx] = 1.0
    RT = np.ascontiguousarray(R.T)

    idn = np.eye(128, dtype=np.float32)
    # causal mask in s^T orientation: keep tq_rel >= tk (upper incl diag)
    mask = np.triu(np.ones((128, 128), dtype=np.float32))
    return cosT, sinT, RT, idn, mask


def _input_maps(x, W_attn, b_attn, W_proj, b_proj):
    cosT, sinT, RT, idn, mask = _host_constants()
    shared = {
        "wa": np.ascontiguousarray(W_attn),
        "bqk": np.ascontiguousarray(b_attn[: 2 * C].reshape(12, 128).T),
        "bv": np.ascontiguousarray(b_attn[2 * C :].reshape(1, C)),
        "wp": np.ascontiguousarray(W_proj),
        "bp": np.ascontiguousarray(b_proj.reshape(1, C)),
        "cosT": cosT,
        "sinT": sinT,
        "rt": RT,
        "idn": idn,
        "mask": mask,
    }
    return [dict(shared, x=np.ascontiguousarray(x[b])) for b in range(B)]


def kernel(x, W_attn, b_attn, W_proj, b_proj):
    global _prog
    from concourse.bass_utils import run_bass_kernel_spmd

    if _prog is None:
        _prog = _build_program()

    x = np.asarray(x, dtype=np.float32)
    W_attn = np.asarray(W_attn, dtype=np.float32)
    b_attn = np.asarray(b_attn, dtype=np.float32)
    W_proj = np.asarray(W_proj, dtype=np.float32)
    b_proj = np.asarray(b_proj, dtype=np.float32)

    in_maps = _input_maps(x, W_attn, b_attn, W_proj, b_proj)
    res = run_bass_kernel_spmd(_prog, in_maps, list(range(N_CORES)))
    out = np.stack([res.results[b]["out"] for b in range(B)], axis=0)
    return out.astype(np.float32)
